# revision 3
# baseline (speedup 1.0000x reference)
"""Sauvola binarization kernel for 8 Trainium2 NeuronCores (data-parallel).

Algorithm (per core, one 1024x1024x3 image):
  gray = RGB dot [0.2989, 0.5870, 0.1140]
  m/m2 = 51x51 reflect-padded box means of gray / gray^2 (via two banded
  fp16 matmul passes on the PE: each pass applies the 51-tap reflect box
  along the partition axis and transposes, so H-pass . W-pass returns to
  the original orientation)
  r = 0.5*(max-min) over ALL images' gray  (AllReduce across the 8 cores)
  thresh = m*(1 + 0.2*(s/r - 1)),  out = (gray > thresh) as f32
"""
import numpy as np

import concourse.bass as bass
import concourse.bass_isa as bass_isa
import concourse.mybir as mybir
import concourse.tile as tile
from concourse.bass_utils import run_bass_kernel_spmd

N_CORES = 8
F = mybir.dt.float32
Hh = mybir.dt.float16
W0, W1, W2 = 0.2989, 0.5870, 0.1140
KS = 0.2
HALF = 25
WINDOWS = [(0, 0, 153), (1, 103, 178), (2, 231, 178), (3, 359, 153), (3, 512, 25),
           (4, 487, 25), (4, 512, 153), (5, 615, 178), (6, 743, 178), (7, 871, 153)]
B0_FIRST, B1_FIRST, B0_LAST, B1_LAST = 0, 4, 5, 9
P0PP = (1.0 - KS) / (2601.0 * W0)
C_BASE = 2.0 * KS / (2601.0 ** 2 * W0)


def _split_multi_waits(nc):
    """walrus here allows one sync wait per instruction; split extras to NOPs."""
    for func in nc.m.functions:
        for bb in func.blocks:
            insts = bb.instructions
            i = 0
            while i < len(insts):
                inst = insts[i]
                si = inst.sync_info
                if si is None or len(si.on_wait) <= 1:
                    i += 1
                    continue
                waits = list(si.on_wait)
                nops = []
                for w in waits[:-1]:
                    nop = mybir.InstNoOp(
                        name=nc.get_next_instruction_name(),
                        sync_info=mybir.SyncInfo(on_wait=[w], on_update=[]),
                        bass_nofuse=True,
                        engine=inst.engine,
                    )
                    nops.append(nop)
                inst.sync_info = mybir.SyncInfo(
                    on_wait=[waits[-1]], on_update=list(si.on_update)
                )
                for k, nop in enumerate(nops):
                    insts.insert(i + k, nop)
                    nc.register_instruction(nop, overwrite=True)
                i += len(nops) + 1


def _build_band_blocks():
    B = np.zeros((1024, 1024), dtype=np.float32)
    idx = np.arange(1024)
    for d in range(-HALF, HALF + 1):
        t = idx + d
        t = np.where(t < 0, -t, t)
        t = np.where(t > 1023, 2046 - t, t)
        np.add.at(B, (idx, t), 1.0)
    blocks = np.zeros((len(WINDOWS), 128, 178), dtype=np.float16)
    for k, (i, c0, ncols) in enumerate(WINDOWS):
        blocks[k, :, :ncols] = B[c0:c0 + ncols, 128 * i:128 * (i + 1)].T
    return blocks


def _build_nc():
    nc = bass.Bass("TRN2", target_bir_lowering=False, debug=False,
                   num_devices=N_CORES)
    x = nc.dram_tensor("x", [1024, 3072], F, kind="ExternalInput")
    band = nc.dram_tensor("band", [len(WINDOWS), 128, 178], Hh, kind="ExternalInput")
    out = nc.dram_tensor("out", [1024, 1024], F, kind="ExternalOutput")

    AluOp = mybir.AluOpType
    Act = mybir.ActivationFunctionType

    with tile.TileContext(nc) as tc:
        with (
            tc.tile_pool(name="consts", bufs=1) as consts,
            tc.tile_pool(name="xin", bufs=3) as xin,
            tc.tile_pool(name="work", bufs=2) as work,
            tc.tile_pool(name="keep", bufs=1) as keep,
            tc.tile_pool(name="grayp", bufs=3) as grayp,
            tc.tile_pool(name="tkeep", bufs=8) as tkeep,
            tc.tile_pool(name="ps", bufs=2, space="PSUM") as ps,
            tc.tile_pool(name="dram", bufs=1, space="DRAM") as dram,
        ):
            # constants
            band_sb = consts.tile([128, len(WINDOWS), 178], Hh)
            nc.sync.dma_start(band_sb[:], band.ap().rearrange("t k n -> k t n"))
            bias_sq = consts.tile([128, 1], F)
            nc.gpsimd.memset(bias_sq[:], -25.5)
            bias_t1 = consts.tile([128, 1], F)
            nc.gpsimd.memset(bias_t1[:], -1300.5)

            xc = x.ap().rearrange("(i p) (j w) -> p i j w", p=128, w=384)
            u2all = keep.tile([128, 8, 8, 128], F)       # gray / W0, all pixels
            accmax = keep.tile([128, 8, 128], Hh)
            accmin = keep.tile([128, 8, 128], Hh)
            ta_tiles, tb_tiles = [], []

            # ---------------- pass 1: per w-chunk j ----------------
            for j in range(8):
                xj = xin.tile([128, 8, 384], F, tag="xj")
                nc.sync.dma_start(xj[:], xc[:, :, j, :])
                s3 = xj[:].rearrange("p i (w c) -> p i w c", c=3)

                u1 = work.tile([128, 8, 128], F, tag="u1")
                nc.vector.scalar_tensor_tensor(
                    u1[:], s3[:, :, :, 1], W1 / W0, s3[:, :, :, 0],
                    op0=AluOp.mult, op1=AluOp.add)
                u2 = u2all[:, :, j, :]
                nc.vector.scalar_tensor_tensor(
                    u2, s3[:, :, :, 2], W2 / W0, u1[:],
                    op0=AluOp.mult, op1=AluOp.add)

                gray = grayp.tile([128, 8, 128], Hh, tag="gray")
                nc.gpsimd.tensor_scalar(gray[:], u2, W0, None, op0=AluOp.mult)
                g2c = grayp.tile([128, 8, 128], Hh, tag="g2c")
                nc.scalar.activation(g2c[:], gray[:], Act.Square,
                                     bias=bias_sq[:], scale=51.0)

                # running min/max of gray (fp16, monotone rounding)
                if j == 0:
                    nc.vector.tensor_copy(accmax[:], gray[:])
                    nc.vector.tensor_copy(accmin[:], gray[:])
                else:
                    nc.vector.tensor_tensor(accmax[:], accmax[:], gray[:], op=AluOp.max)
                    nc.vector.tensor_tensor(accmin[:], accmin[:], gray[:], op=AluOp.min)

                # P1 banded matmuls: out[w, hp] += gray[h, w] * B[hp, h]
                pa = ps.tile([128, 1024], F, tag="A")
                pb = ps.tile([128, 1024], F, tag="B")
                for stat, (src, pt) in enumerate(((gray, pa), (g2c, pb))):
                    for k, (i, c0, ncols) in enumerate(WINDOWS):
                        nc.tensor.matmul(
                            pt[:, c0:c0 + ncols], src[:, i, :],
                            band_sb[:, k, :ncols],
                            start=(k in (B0_FIRST, B1_FIRST)),
                            stop=(k in (B0_LAST, B1_LAST)))
                ta = tkeep.tile([128, 1024], Hh, tag="ta")
                nc.scalar.copy(ta[:], pa[:])
                tb = tkeep.tile([128, 1024], Hh, tag="tb")
                nc.scalar.copy(tb[:], pb[:])
                ta_tiles.append(ta)
                tb_tiles.append(tb)

            # ---------------- global r via AllReduce(max) ----------------
            rmax = consts.tile([128, 1], F)
            rminn = consts.tile([128, 1], F)
            nc.vector.tensor_reduce(rmax[:], accmax[:].rearrange("p a b -> p (a b)"),
                                    mybir.AxisListType.X, AluOp.max)
            nc.vector.tensor_reduce(rminn[:], accmin[:].rearrange("p a b -> p (a b)"),
                                    mybir.AxisListType.X, AluOp.min)
            nc.vector.tensor_scalar(rminn[:], rminn[:], -1.0, None, op0=AluOp.mult)
            gmax = consts.tile([1, 1], F)
            gminn = consts.tile([1, 1], F)
            nc.gpsimd.tensor_reduce(gmax[:], rmax[:], mybir.AxisListType.C, AluOp.max)
            nc.gpsimd.tensor_reduce(gminn[:], rminn[:], mybir.AxisListType.C, AluOp.max)
            mm_sb = consts.tile([1, 2], F)
            nc.vector.tensor_copy(mm_sb[:, 0:1], gmax[:])
            nc.vector.tensor_copy(mm_sb[:, 1:2], gminn[:])
            mm_in = dram.tile([1, 2], F)
            mm_sh = dram.tile([1, 2], F, addr_space="Shared")
            nc.sync.dma_start(mm_in[:], mm_sb[:])
            nc.gpsimd.collective_compute(
                "AllReduce", AluOp.max,
                replica_groups=[list(range(N_CORES))],
                ins=[mm_in.opt()], outs=[mm_sh.opt()])
            # broadcast-read the 8-byte result into all 128 partitions
            mm_b = consts.tile([128, 2], F)
            nc.sync.dma_start(mm_b[:], mm_sh[:].to_broadcast((128, 2)))
            rsum = consts.tile([128, 1], F)     # gmax - gmin = 2r (per partition)
            nc.vector.tensor_reduce(rsum[:], mm_b[:], mybir.AxisListType.X, AluOp.add)
            crec = consts.tile([128, 1], F)
            nc.vector.reciprocal(crec[:], rsum[:])
            c1 = consts.tile([128, 1], F)
            nc.vector.tensor_scalar(c1[:], crec[:], C_BASE, None, op0=AluOp.mult)
            ccvec = consts.tile([128, 1], F)
            nc.vector.tensor_tensor(ccvec[:], c1[:], c1[:], op=AluOp.mult)

            # ---------------- pass 2 + threshold per hp-chunk m ----------------
            for m in range(8):
                qa = ps.tile([128, 1024], F, tag="A")
                qb = ps.tile([128, 1024], F, tag="B")
                for src_tiles, pt in ((ta_tiles, qa), (tb_tiles, qb)):
                    for k, (jj, c0, ncols) in enumerate(WINDOWS):
                        nc.tensor.matmul(
                            pt[:, c0:c0 + ncols],
                            src_tiles[jj][:, 128 * m:128 * (m + 1)],
                            band_sb[:, k, :ncols],
                            start=(k in (B0_FIRST, B1_FIRST)),
                            stop=(k in (B0_LAST, B1_LAST)))
                qa3 = qa[:].rearrange("p (a b) -> p a b", b=128)
                qb3 = qb[:].rearrange("p (a b) -> p a b", b=128)
                t1 = work.tile([128, 8, 128], F, tag="t1")
                nc.scalar.activation(t1[:], qa3, Act.Square, bias=bias_t1[:], scale=1.0)
                t2 = work.tile([128, 8, 128], F, tag="t2")
                nc.vector.scalar_tensor_tensor(
                    t2[:], t1[:], -1.0, qb3, op0=AluOp.mult, op1=AluOp.add)
                sin = work.tile([128, 8, 128], F, tag="sin")
                nc.gpsimd.tensor_scalar(sin[:], t2[:], ccvec[:], 0.0,
                                        op0=AluOp.mult, op1=AluOp.max)
                s0 = work.tile([128, 8, 128], F, tag="s0")
                nc.scalar.activation(s0[:], sin[:], Act.Sqrt)
                q = work.tile([128, 8, 128], F, tag="q")
                nc.vector.scalar_tensor_tensor(
                    q[:], s0[:], P0PP, qa3, op0=AluOp.add, op1=AluOp.mult)
                dmask = work.tile([128, 8, 128], F, tag="dmask")
                nc.gpsimd.tensor_tensor(dmask[:], u2all[:, m, :, :], q[:],
                                        op=AluOp.subtract)
                mask = work.tile([128, 8, 128], F, tag="mask")
                nc.gpsimd.tensor_scalar(mask[:], dmask[:], 0.0, None,
                                        op0=AluOp.is_gt)
                out_m = out.ap().rearrange("(m p) (a b) -> m p a b", p=128, b=128)[m]
                nc.sync.dma_start(out_m, mask[:])

    _split_multi_waits(nc)
    return nc


_CACHE = {}


def _get_nc():
    if "nc" not in _CACHE:
        _CACHE["nc"] = _build_nc()
        _CACHE["band"] = _build_band_blocks()
    return _CACHE["nc"], _CACHE["band"]


def kernel(inputs: np.ndarray) -> np.ndarray:
    nc, band = _get_nc()
    x = np.asarray(inputs, dtype=np.float32)
    in_maps = [
        {"x": np.ascontiguousarray(x[c].reshape(1024, 3072)), "band": band}
        for c in range(N_CORES)
    ]
    res = run_bass_kernel_spmd(nc, in_maps, list(range(N_CORES)))
    masks = [res.results[c]["out"] for c in range(N_CORES)]
    return np.stack(masks)[..., None].astype(np.float32)
